# revision 42
# baseline (speedup 1.0000x reference)
"""Trainium2 Bass kernel for nn_MultiHeadDynamics.

Computation (per sample row x of state, s of signal):
    heads   = x.reshape(H, DH)                      # H=16, DH=256
    A_h     = U_h @ V_h + diag(d_h)                 # (DH, DH) per head
    lin     = heads @ A_h^T
    c       = heads - mean_dh(heads)
    drift   = lin + cs * c^3 + s
    out     = x + DT*(1+cp)*drift - (DT*cp/H) * sum_h(drift_h)

Folding:  beta = DT*(1+cp);  gp = DT*cp/(H*beta);  q = cbrt(beta*cs)
    D'      = beta*drift
    out     = x + D' - gp * sum_h(D'_h)

Host-side precompute (weight preprocessing, tiny):
    ATm[p, h, k, e] = beta * (A_h[e, d] with d = k*128+p)   as bf16
    Cq[p, k, e]     = q * (I - J/DH)[d, e]                  as bf16
so that on-device, with xT = per-128-chunk PE transpose of x:
    beta*lin (head h) = sum_k  xT_chunk(2h+k).T @ ATm[:, h, k, :]
    ch = q*(x - mean)  (head h) = sum_k  xT_chunk(2h+k).T @ Cq[:, k, :]
    c3 = ch*ch*ch  (fp16, DVE 2x)
    D' in PSUM = beta*lin  (+ c3 + beta*s folded via fp16 identity matmuls)

Sharding: batch B=8192 split across 8 cores (1024 rows each), params
replicated. Per core, rows are processed as 8 tiles of [128, 4096].
"""

import sys

for _p in ("/opt/trn_rl_repo",):
    if _p not in sys.path:
        sys.path.insert(0, _p)

from contextlib import ExitStack

import numpy as np
import ml_dtypes

import concourse.bass as bass
import concourse.tile as tile
from concourse import bacc, mybir
from concourse.bass_utils import run_bass_kernel_spmd
from concourse.masks import make_identity

F32 = mybir.dt.float32
BF16 = mybir.dt.bfloat16
FP16 = mybir.dt.float16
AOP = mybir.AluOpType

# Problem constants (full-input shapes; hardcoded per the task contract).
B = 8192
D = 4096
H = 16
DH = 256
R = 64
DT = 0.05
NCORES = 8
BS = B // NCORES          # rows per core = 1024
P = 128                   # partitions
NT = BS // P              # row tiles per core = 8
NCH = D // P              # 128-wide column chunks per row tile = 32

# Columns of the final fp32 (x + dd) pass handled by DVE; the rest on
# GpSimd.
FINAL_DVE_COLS = 1024


def _emit(tc: tile.TileContext, aps: dict, beta: float, gp: float):
    nc = tc.nc

    state = aps["state"]
    signal = aps["signal"]
    AT_d = aps["ATm"]
    Cq_d = aps["Cq"]
    out_d = aps["out"]

    with ExitStack() as ctx:
        consts = ctx.enter_context(tc.tile_pool(name="consts", bufs=1))

        ident = consts.tile([P, P], F32, tag="ident")
        make_identity(nc, ident)
        identh = consts.tile([P, P], FP16, tag="identh")
        make_identity(nc, identh)

        # --- main loop pools ---
        xhp = ctx.enter_context(tc.tile_pool(name="xhp", bufs=7))
        sp = ctx.enter_context(tc.tile_pool(name="sp", bufs=3))
        sbp = ctx.enter_context(tc.tile_pool(name="sbp", bufs=2))
        hp = ctx.enter_context(tc.tile_pool(name="hp", bufs=2))
        chp = ctx.enter_context(tc.tile_pool(name="chp", bufs=2))
        c2p = ctx.enter_context(tc.tile_pool(name="c2p", bufs=2))
        c3p = ctx.enter_context(tc.tile_pool(name="c3p", bufs=2))
        ps_tp = ctx.enter_context(tc.tile_pool(name="ps_tp", bufs=2, space="PSUM"))
        ps_ch = ctx.enter_context(tc.tile_pool(name="ps_ch", bufs=2, space="PSUM"))
        ps_lin = ctx.enter_context(tc.tile_pool(name="ps_lin", bufs=2, space="PSUM"))

        AT = consts.tile([P, H * 2 * DH], BF16, tag="AT")
        Cq = consts.tile([P, 2 * DH], BF16, tag="Cq")
        HD = D // 2

        def f_dma(it, st):
            """Input DMA triggers, all on the idle Sync engine FIFO."""
            r0 = it * P
            # x as two half-tiles from a deep pool: finer-grained buffer
            # recycling so the input prefetch isn't chained to whole-tile
            # completion of the final adds.
            x0 = st["x0"] = xhp.tile([P, HD], F32, tag="xh", name="x0")
            x1 = st["x1"] = xhp.tile([P, HD], F32, tag="xh", name="x1")
            s_t = st["s"] = sp.tile([P, D], F32, tag="s", name="s_t")
            nc.sync.dma_start(out=x0, in_=state[r0:r0 + P, 0:HD])
            nc.sync.dma_start(out=s_t[:, 0:HD], in_=signal[r0:r0 + P, 0:HD])
            nc.sync.dma_start(out=x1, in_=state[r0:r0 + P, HD:D])
            nc.sync.dma_start(out=s_t[:, HD:D], in_=signal[r0:r0 + P, HD:D])
            st["sb"] = sbp.tile([P, D], FP16, tag="sb", name="sb_t")
            if it == 0:
                # Consts ride behind the first input tile.
                nc.sync.dma_start(out=Cq, in_=Cq_d)
                nc.sync.dma_start(out=AT, in_=AT_d)
                # Warm the PE's HAM clock gate while the first DMA
                # streams so real matmuls run at 2.4 GHz from the start.
                warm = ps_tp.tile([P, 4 * P], F32, tag="tp_ps", name="warm")
                for w in range(16):
                    nc.tensor.matmul(
                        warm[:, (w % 4) * P:(w % 4 + 1) * P], lhsT=ident,
                        rhs=ident, is_transpose=True, skip_group_check=True,
                    )
            st["hT"] = hp.tile([P, NCH, P], BF16, tag="hT", name="hT")
            st["chs"] = chp.tile([P, D], FP16, tag="chs", name="chs")
            st["c2"] = c2p.tile([P, D], FP16, tag="c2", name="c2_t")
            st["c3"] = c3p.tile([P, D], FP16, tag="c3", name="c3_t")

        def _transp_group(st, tg):
            xh = st["x0"] if tg < 4 else st["x1"]
            hT = st["hT"]
            base = 0 if tg < 4 else HD
            tp_ps = ps_tp.tile([P, 4 * P], F32, tag="tp_ps", name="tp_ps")
            for c in range(4):
                j = tg * 4 + c
                nc.tensor.transpose(
                    tp_ps[:, c * P:(c + 1) * P],
                    xh[:, j * P - base:(j + 1) * P - base], ident,
                )
            nc.scalar.copy(
                out=hT[:, tg * 4:(tg + 1) * 4, :].rearrange(
                    "p a b -> p (a b)"),
                in_=tp_ps,
            )

        def _ch_group(st, pr):
            # heads 2*pr, 2*pr+1 -> chunks 4*pr .. 4*pr+3
            hT, chs = st["hT"], st["chs"]
            ch_ps = ps_ch.tile([P, 2 * DH], F32, tag="ch_ps", name="ch_ps")
            # NOTE: start=True clears has_written for the WHOLE bank,
            # so only the first matmul touching a bank may set it.
            for hh in range(2):
                h = pr * 2 + hh
                for k in range(2):
                    nc.tensor.matmul(
                        ch_ps[:, hh * DH:(hh + 1) * DH],
                        lhsT=hT[:, 2 * h + k, :],
                        rhs=Cq[:, k * DH:(k + 1) * DH],
                        start=(hh == 0 and k == 0),
                        stop=(hh == 1 and k == 1),
                        skip_group_check=True,
                    )
            nc.scalar.copy(
                out=chs[:, pr * 2 * DH:(pr + 1) * 2 * DH], in_=ch_ps,
            )

        def _cube_half(st, half):
            # c3 = (q*c)^3 = beta*cs*c^3, fp16 on DVE (2x)
            chs, c2_t, c3_t = st["chs"], st["c2"], st["c3"]
            sl = slice(half * HD, (half + 1) * HD)
            nc.vector.tensor_mul(c2_t[:, sl], chs[:, sl], chs[:, sl])
            nc.vector.tensor_mul(c3_t[:, sl], c2_t[:, sl], chs[:, sl])

        def _sb_half(st, half):
            # beta*s in fp16 (fold target for the drift PSUM); early and
            # per-half so s_t is dead as soon as possible (its buffer
            # doubles as the out buffer).
            hs = slice(half * HD, (half + 1) * HD)
            nc.vector.tensor_scalar(
                out=st["sb"][:, hs], in0=st["s"][:, hs], scalar1=beta,
                scalar2=None, op0=AOP.mult,
            )

        def fA(it, st):
            """First half of the tile front: transposes + heads 0-3."""
            _transp_group(st, 0)
            _transp_group(st, 1)
            _ch_group(st, 0)
            _transp_group(st, 2)
            _sb_half(st, 0)
            _ch_group(st, 1)
            _transp_group(st, 3)

        def fB(it, st):
            """Second half of the tile front: heads 4-15 + cubic."""
            _ch_group(st, 2)
            _transp_group(st, 4)
            _ch_group(st, 3)
            _cube_half(st, 0)
            _sb_half(st, 1)
            _transp_group(st, 5)
            _ch_group(st, 4)
            _transp_group(st, 6)
            _ch_group(st, 5)
            _transp_group(st, 7)
            _ch_group(st, 6)
            _ch_group(st, 7)
            _cube_half(st, 1)

        def _lin_group(st, g):
            # Per-head-group matmuls into PSUM (beta*lin), then fold
            # c3 and beta*s into the same banks via fp16 identity
            # matmuls -> PSUM holds D' = beta*(lin + cs*c^3 + s).
            hT, sb_t, c3_t = st["hT"], st["sb"], st["c3"]
            drs = st["chs"]  # drs reuses the dead chs buffer
            l_ps = ps_lin.tile([P, 4 * DH], F32, tag="l_ps", name="l_ps")
            # one start=True per bank (hh 0 and 2); everything else
            # relies on per-element has_written accumulate-vs-write.
            for hh in range(4):
                h = g * 4 + hh
                for k in range(2):
                    nc.tensor.matmul(
                        l_ps[:, hh * DH:(hh + 1) * DH],
                        lhsT=hT[:, 2 * h + k, :],
                        rhs=AT[:, (h * 2 + k) * DH:(h * 2 + k + 1) * DH],
                        start=(hh % 2 == 0 and k == 0), stop=False,
                        skip_group_check=True,
                    )
            for half in range(2):
                psl = slice(half * 2 * DH, (half + 1) * 2 * DH)
                csl = slice(g * 4 * DH + half * 2 * DH,
                            g * 4 * DH + (half + 1) * 2 * DH)
                nc.tensor.matmul(
                    l_ps[:, psl], lhsT=identh, rhs=c3_t[:, csl],
                    start=False, stop=False, skip_group_check=True,
                )
                nc.tensor.matmul(
                    l_ps[:, psl], lhsT=identh, rhs=sb_t[:, csl],
                    start=False, stop=True, skip_group_check=True,
                )
            nc.scalar.copy(out=drs[:, g * 4 * DH:(g + 1) * 4 * DH],
                           in_=l_ps)

        def bA(it, st):
            _lin_group(st, 0)
            _lin_group(st, 1)

        def bB(it, st):
            _lin_group(st, 2)
            _lin_group(st, 3)

        def bC(it, st):
            """Head-sum coupling, final add, writeback."""
            r0 = it * P
            sb_t, c3_t, c2_t = st["sb"], st["c3"], st["c2"]
            x0, x1 = st["x0"], st["x1"]
            drs = st["chs"]

            # head-sum tree, flat contiguous halves (order-independent
            # sum). Scratch lives in the dead c2 buffer (cube is done);
            # raw (unscaled) head-sum lands in dead c3.
            t8 = c2_t[:, 0:D // 2]
            nc.vector.tensor_add(t8, drs[:, 0:D // 2], drs[:, D // 2:D])
            t4 = c2_t[:, D // 2:D // 2 + D // 4]
            nc.vector.tensor_add(t4, t8[:, 0:D // 4], t8[:, D // 4:D // 2])
            t2r = c2_t[:, 3 * D // 4:3 * D // 4 + D // 8]
            nc.vector.tensor_add(t2r, t4[:, 0:D // 8], t4[:, D // 8:D // 4])
            # mlt = 4 side-by-side copies of -gp*sum_h(D') in dead c3
            mlt = c3_t[:, 0:4 * DH]
            nc.vector.tensor_add(mlt[:, 0:DH], t2r[:, 0:DH],
                                 t2r[:, DH:2 * DH])
            nc.vector.tensor_scalar_mul(mlt[:, 0:DH], mlt[:, 0:DH], -gp)
            nc.vector.tensor_copy(mlt[:, DH:2 * DH], mlt[:, 0:DH])
            nc.vector.tensor_copy(mlt[:, 2 * DH:4 * DH], mlt[:, 0:2 * DH])

            # dd = D' + mlt into c2's dead tree-scratch regions (fp16
            # TT stays in DVE 2x mode; broadcast APs would drop to 1x)
            dd = c2_t

            def dd_g(g):
                gsl = slice(g * 4 * DH, (g + 1) * 4 * DH)
                nc.vector.tensor_add(dd[:, gsl], drs[:, gsl], mlt)

            # out = x + dd into the dead s buffer; GpSimd segments get
            # their dd inputs first (its fp32 adds are the slow tail
            # that gates x recycling).
            o_t = st["s"]
            if it == NT - 1:
                for g in range(4):
                    dd_g(g)
                nc.vector.tensor_add(o_t[:, 0:HD], x0, dd[:, 0:HD])
                nc.vector.tensor_add(
                    o_t[:, HD:HD + 1024], x1[:, 0:1024],
                    dd[:, HD:HD + 1024])
                nc.gpsimd.tensor_add(
                    o_t[:, HD + 1024:D], x1[:, 1024:HD],
                    dd[:, HD + 1024:D])
            else:
                dd_g(1)
                nc.gpsimd.tensor_add(
                    o_t[:, 1024:HD], x0[:, 1024:HD], dd[:, 1024:HD])
                dd_g(2)
                dd_g(3)
                nc.gpsimd.tensor_add(o_t[:, HD:D], x1, dd[:, HD:D])
                dd_g(0)
                nc.vector.tensor_add(
                    o_t[:, 0:1024], x0[:, 0:1024], dd[:, 0:1024])
            # writeback on the GpSimd (SWDGE) queue: a trigger waiting
            # on the final adds would head-block a busier engine FIFO.
            nc.gpsimd.dma_start(out=out_d[r0:r0 + P, 0:HD],
                                in_=o_t[:, 0:HD])
            nc.gpsimd.dma_start(out=out_d[r0:r0 + P, HD:D],
                                in_=o_t[:, HD:D])

        # Software pipeline: F(0) F(1) B(0) F(2) B(1) ... B(NT-1).
        # Each engine always has a tile of independent work queued, so
        # the PE->ACT->DVE->PE dependency ring never head-blocks a FIFO.
        prev = None
        pend = []
        for it in range(NT):
            cur = {}
            f_dma(it, cur)
            fA(it, cur)
            fB(it, cur)
            pend.append((it, cur))
            if len(pend) > 1:
                bit, bst = pend.pop(0)
                bA(bit, bst)
                bB(bit, bst)
                bC(bit, bst)
        bit, bst = pend.pop(0)
        bA(bit, bst)
        bB(bit, bst)
        bC(bit, bst)


_CACHE: dict = {}


def _build(beta: float, gp: float) -> bass.Bass:
    key = (float(beta), float(gp), FINAL_DVE_COLS)
    if key in _CACHE:
        return _CACHE[key]
    nc = bacc.Bacc("TRN2", target_bir_lowering=False, debug=False)
    aps = {
        "state": nc.dram_tensor("state", [BS, D], F32, kind="ExternalInput").ap(),
        "signal": nc.dram_tensor("signal", [BS, D], F32, kind="ExternalInput").ap(),
        "ATm": nc.dram_tensor("ATm", [P, H * 2 * DH], BF16, kind="ExternalInput").ap(),
        "Cq": nc.dram_tensor("Cq", [P, 2 * DH], BF16, kind="ExternalInput").ap(),
        "out": nc.dram_tensor("out", [BS, D], F32, kind="ExternalOutput").ap(),
    }
    with tile.TileContext(nc) as tc:
        _emit(tc, aps, float(beta), float(gp))
    nc.compile()
    _CACHE[key] = nc
    return nc


def _host_params(U, V, diag, cubic_scale, coupling):
    """Fold the tiny per-head params into the matmul operand layouts."""
    beta = DT * (1.0 + coupling)
    gp = DT * coupling / (H * beta)
    q = (beta * cubic_scale) ** (1.0 / 3.0)

    # Reference: A[h, d1, e1] = sum_r U[h,d1,r] V[h,r,e1]; in the lin
    # einsum A is indexed [h, e, d] -> M[h, d, e] := A[h, e, d] (+ diag).
    A = np.einsum("hdr,hre->hde", U, V).astype(np.float32)
    M = np.ascontiguousarray(np.transpose(A, (0, 2, 1)))
    idx = np.arange(DH)
    M[:, idx, idx] += diag
    ATm = (beta * M).reshape(H, 2, P, DH).transpose(2, 0, 1, 3)
    ATm = np.ascontiguousarray(ATm.reshape(P, H * 2 * DH)).astype(
        ml_dtypes.bfloat16
    )

    Cmat = q * (np.eye(DH, dtype=np.float32) - 1.0 / DH)
    Cq = Cmat.reshape(2, P, DH).transpose(1, 0, 2)
    Cq = np.ascontiguousarray(Cq.reshape(P, 2 * DH)).astype(ml_dtypes.bfloat16)
    return beta, gp, ATm, Cq


def run(state, signal, U, V, diag, cubic_scale, coupling, trace=False):
    state = np.ascontiguousarray(np.asarray(state, dtype=np.float32))
    signal = np.ascontiguousarray(np.asarray(signal, dtype=np.float32))
    U = np.asarray(U, dtype=np.float32)
    V = np.asarray(V, dtype=np.float32)
    diag = np.asarray(diag, dtype=np.float32)

    beta, gp, ATm, Cq = _host_params(U, V, diag, float(cubic_scale),
                                     float(coupling))
    nc = _build(beta, gp)
    in_maps = []
    for i in range(NCORES):
        sl = slice(i * BS, (i + 1) * BS)
        in_maps.append({
            "state": state[sl], "signal": signal[sl],
            "ATm": ATm, "Cq": Cq,
        })
    res = run_bass_kernel_spmd(nc, in_maps, list(range(NCORES)), trace=trace)
    out = np.concatenate([res.results[i]["out"] for i in range(NCORES)], axis=0)
    return out, res


def kernel(state, signal, U, V, diag, cubic_scale, coupling) -> np.ndarray:
    out, _ = run(state, signal, U, V, diag, cubic_scale, coupling, trace=False)
    return out
